# revision 23
# baseline (speedup 1.0000x reference)
"""Multi-head attention (B=4, S=2048, D=1024, H=16, dk=64) on 8 TRN2 NeuronCores.

Sharding: core c = (batch b = c//2, head-group g = c%2 of 8 heads).
Each core computes its head-group's attention output and the partial output
projection (Wo rows for its heads); the host sums the two partials per batch
and adds the (folded) output bias.

All-bf16 pipeline (fp8 was tried and abandoned: ~2.7% per-element
quantization noise does NOT average down through the diffuse softmax -- the
attention output's own magnitude shrinks identically -- so any fp8 stage
busts the 2e-2 budget; fp8 DoubleRow also measured no faster than bf16,
and fp8 matmuls pay a serialized ~88ns LDWEIGHTS that bf16's FWL hides).

Per-core stages:
  - Q/K/V projections: bf16 matmuls, PSUM drained with bias-add to bf16.
  - scores: bf16 64-row matmuls; head pairs on PE row groups 0/64 run
    concurrently. PSUM holds raw s = q.k (std ~8); exp applies the 1/8.
  - exp, split across engines (elementwise PSUM-drain rate is the second
    wall after the PE):
      'A' tiles: Scalar exact exp(s/8) -> bf16.
      'D' tiles: DVE 4-op accurate fast-exp (max err ~0.9%, median 0.2%):
        1. y = s*C0 + MAGIC  (magic-add rounds the bf16-bit index i into
           the f32 mantissa; MAGIC = 1.5*2^23 + 16256)
        2. phi = bits(y) & 127             (mantissa phase, int->float)
        3. q = (phi - 128) * phi           (scalar_tensor_tensor)
        4. es_bits = (q*PA + K) + y        (affine_then_add custom op;
           K = -1.5*2^23; PA*phi*(phi-128) = A*(phi-64)^2 - 10.98 is the
           2^f-vs-1+f mantissa correction; u16 out saturates underflow to 0)
  - attn @ V: bf16 [65,512] matmuls; V augmented with a trailing ones
    column so PSUM row 64 accumulates the softmax denominators.
  - normalize: r row (psum partition 64) copied to partition 0 on DVE,
    reciprocal_approx_fast, GpSimd broadcast, DVE multiply -> bf16 O^T.
  - output projection: bf16 matmuls; y drained to bf16 on Scalar, DMA out.
PSUM: psS pool 3x[128,1024] f32 (6 banks) backs the scores/exp pipeline,
the projection accumulators (two halves each) and the outproj psy; psU
2x[65,512] holds the U/denominator accumulators of the active unit.
V-bias and output bias are folded on the host: softmax rows sum to 1, so
bv contributes bv_cat @ Wo + bo to every row.
"""

import numpy as np
import ml_dtypes

B, S, D = 4, 2048, 1024
H, DK = 16, 64
LH = 8                 # heads per core
HK = LH * DK           # 512 (local concat dim)
BLK = 512              # Sq block size
NB = S // BLK          # 4
ST = S // 128          # 16 Skv tiles
KT = D // 128          # 8 contraction tiles over D
MT = HK // 128         # 4 m-tiles over local heads

# exp engine per kv tile i ('A' Scalar exact exp, 'D' DVE 4-op fast-exp),
# alternating by unit parity: A 216 / D 40 of 256 tiles total. D tiles lead
# each unit so their 4-op chains mature before the U matmuls (ULAG) need them.
EXP_PATTERNS = ["DDDAAAAAAAAAAAAA",  # A=13, D=3
                "DDAAAAAAAAAAAAAA"]  # A=14, D=2

# fast-exp consts: bf16 bits of exp(s/8): i = (s/8)*128*log2(e) + 16256
FE_SCALE = 184.6650053442343 / 8.0
FE_MAGIC = 12582912.0 + 16256.0
FE_PA = 10.9834 / 4096.0
FE_K = -12582912.0

_CACHE = {}


def _build_program():
    from contextlib import ExitStack
    import concourse.tile as tile
    from concourse import bacc, mybir

    f32 = mybir.dt.float32
    bf16 = mybir.dt.bfloat16
    i32 = mybir.dt.int32
    u16 = mybir.dt.uint16
    Exp = mybir.ActivationFunctionType.Exp
    Copy = mybir.ActivationFunctionType.Copy
    Mult = mybir.AluOpType.mult
    Add = mybir.AluOpType.add
    Band = mybir.AluOpType.bitwise_and
    Sub = mybir.AluOpType.subtract

    nc = bacc.Bacc("TRN2", target_bir_lowering=False, debug=False, num_devices=8)

    xq_d = nc.dram_tensor("xq_t", [D, S], bf16, kind="ExternalInput")
    xk_d = nc.dram_tensor("xk_t", [D, S], bf16, kind="ExternalInput")
    xv_d = nc.dram_tensor("xv_t", [D, S], bf16, kind="ExternalInput")
    wq_d = nc.dram_tensor("wq", [D, HK], bf16, kind="ExternalInput")
    wk_d = nc.dram_tensor("wk", [D, HK], bf16, kind="ExternalInput")
    wv_d = nc.dram_tensor("wv", [D, HK], bf16, kind="ExternalInput")
    wo_d = nc.dram_tensor("wo", [HK, D], bf16, kind="ExternalInput")
    bq_d = nc.dram_tensor("bq2", [128, MT], f32, kind="ExternalInput")
    bk_d = nc.dram_tensor("bk2", [128, MT], f32, kind="ExternalInput")
    y_d = nc.dram_tensor("y_t", [D, S], bf16, kind="ExternalOutput")

    with tile.TileContext(nc) as tc, ExitStack() as ctx:
        wpool = ctx.enter_context(tc.tile_pool(name="w", bufs=2))
        big = ctx.enter_context(tc.tile_pool(name="big", bufs=1))
        xs = ctx.enter_context(tc.tile_pool(name="xs", bufs=3))
        es_pool = ctx.enter_context(tc.tile_pool(name="es", bufs=6))
        fe_pool = ctx.enter_context(tc.tile_pool(name="fe", bufs=6))
        ot_pool = ctx.enter_context(tc.tile_pool(name="ot", bufs=2))
        rpool = ctx.enter_context(tc.tile_pool(name="r", bufs=2))
        upool = ctx.enter_context(tc.tile_pool(name="u", bufs=2))
        ypool = ctx.enter_context(tc.tile_pool(name="y", bufs=2))
        # PSUM: psS 3x[128,1024] f32 (6 banks) + psU 2x[65,512] (2) = 8
        psS = ctx.enter_context(tc.tile_pool(name="psS", bufs=3, space="PSUM"))
        psU = ctx.enter_context(tc.tile_pool(name="psU", bufs=2, space="PSUM"))

        bq_sb = big.tile([128, MT], f32)
        bk_sb = big.tile([128, MT], f32)
        nc.sync.dma_start(bq_sb[:], bq_d[:])
        nc.sync.dma_start(bk_sb[:], bk_d[:])
        qt = big.tile([128, MT, S], bf16)
        kt_ = big.tile([128, MT, S], bf16)
        # vaug[p, st, h, 0:DK] = V, [.., DK] = 1 (denominator column)
        vaug = big.tile([128, ST, LH, DK + 1], bf16)
        nc.vector.memset(vaug[:, :, :, :].bitcast(u16), 0x3F80)  # bf16 1.0

        def make_proj(x_dram, w_dram, bias_sb, dst, wname):
            """Returns (dma_block(j), mm_block(j)) emitters; loads weights now."""
            w_sb = wpool.tile([128, KT, HK], bf16, tag="w", name=f"w_{wname}")
            nc.sync.dma_start(w_sb[:], w_dram.ap().rearrange("(kt p) m -> p kt m", p=128))
            xts = {}

            def dma_block(j):
                xt = xs.tile([128, KT, BLK], bf16, tag="xs", name=f"xt_{wname}{j}")
                nc.sync.dma_start(
                    xt[:],
                    x_dram.ap()[:, j * BLK : (j + 1) * BLK]
                    .rearrange("(kt p) s -> p kt s", p=128),
                )
                xts[j] = xt

            def mm_block(j):
                xt = xts.pop(j)
                if wname == "v":
                    # V: x as stationary; drain [128, hk] kv-rows to vaug
                    for qp in range(2):
                        pt = psS.tile([128, 2 * BLK], f32, tag="psS",
                                      name=f"pp_v{j}_{qp}")
                        for kt in range(KT):
                            for t in range(2):
                                q = 2 * qp + t
                                nc.tensor.matmul(
                                    pt[:, t * BLK : (t + 1) * BLK],
                                    xt[:, kt, q * 128 : (q + 1) * 128],
                                    w_sb[:, kt, :],
                                    start=(kt == 0),
                                    stop=(kt == KT - 1),
                                    skip_group_check=True,
                                )
                        for t in range(2):
                            st = j * 4 + 2 * qp + t
                            nc.scalar.activation(
                                vaug[:, st, :, 0:DK],
                                pt[:, t * BLK : (t + 1) * BLK]
                                .rearrange("p (h k) -> p h k", h=LH),
                                Copy,
                            )
                else:
                    for mp in range(2):  # two mt-pairs share one psS slot
                        pt = psS.tile([128, 2 * BLK], f32, tag="psS",
                                      name=f"pp_{wname}{j}_{mp}")
                        for kt in range(KT):
                            for t in range(2):
                                mt = 2 * mp + t
                                nc.tensor.matmul(
                                    pt[:, t * BLK : (t + 1) * BLK],
                                    w_sb[:, kt, mt * 128 : (mt + 1) * 128],
                                    xt[:, kt, :],
                                    start=(kt == 0),
                                    stop=(kt == KT - 1),
                                    skip_group_check=True,
                                )
                        for t in range(2):
                            mt = 2 * mp + t
                            nc.vector.tensor_scalar_add(
                                dst[:, mt, j * BLK : (j + 1) * BLK],
                                pt[:, t * BLK : (t + 1) * BLK],
                                bias_sb[:, mt : mt + 1],
                            )

            return dma_block, mm_block

        # K fully first (scores of any unit span all kv blocks), then V/Q
        # block 0; remaining V/Q blocks are woven into the first attention
        # units so the exp engines start ~45us earlier.
        k_dma, k_mm = make_proj(xk_d, wk_d, bk_sb, kt_, "k")
        for j in range(NB):
            k_dma(j)
            k_mm(j)
        v_dma, v_mm = make_proj(xv_d, wv_d, None, None, "v")
        q_dma, q_mm = make_proj(xq_d, wq_d, bq_sb, qt, "q")
        v_dma(0)
        v_mm(0)
        q_dma(0)
        q_mm(0)

        state = {}

        def load_wo():
            wo_sb = wpool.tile([128, MT, D], bf16, tag="w")
            nc.sync.dma_start(
                wo_sb[:], wo_d.ap().rearrange("(kt p) m -> p kt m", p=128)
            )
            state["wo_sb"] = wo_sb

        def emit_outproj(jprev, ot_prev, mos):
            wo_sb = state["wo_sb"]
            for mo in mos:
                psy = psS.tile([128, 2 * BLK], f32, tag="psS",
                               name=f"psy{jprev}_{mo}")[:, 0:BLK]
                for kt in range(MT):
                    nc.tensor.matmul(
                        psy,
                        wo_sb[:, kt, mo * 128 : (mo + 1) * 128],
                        ot_prev[:, kt, :],
                        start=(kt == 0),
                        stop=(kt == MT - 1),
                        skip_group_check=True,
                    )
                ysb = ypool.tile([128, BLK], bf16, tag="y", name=f"ysb{jprev}_{mo}")
                nc.scalar.activation(ysb[:], psy, Copy)
                nc.sync.dma_start(
                    y_d[mo * 128 : (mo + 1) * 128,
                        jprev * BLK : (jprev + 1) * BLK], ysb[:]
                )

        ULAG = 4  # U matmuls trail the exp stream so D-tile chains mature
        TLAG = 2  # chain tails fire 2 tiles ahead of their U consumer
        # normalize/outproj of a unit are emitted early in the NEXT unit so
        # they never sit ahead of the PSUM-freeing exp ops on the DVE queue
        deferred = {"norm": None, "outproj": None}
        ot_prev = None
        for j in range(NB):
            otj = ot_pool.tile([128, MT, BLK], bf16)
            for hp in range(LH // 2):
                mt = hp
                unit = j * 4 + hp
                pattern = EXP_PATTERNS[unit % 2]
                if j == 0 and hp == 0:
                    hooks = {1: lambda: v_dma(1), 3: lambda: v_mm(1),
                             5: lambda: v_dma(2), 7: lambda: v_mm(2),
                             9: lambda: v_dma(3), 11: lambda: v_mm(3)}
                elif j == 0:
                    hooks = {3: lambda jj=hp: q_dma(jj),
                             7: lambda jj=hp: q_mm(jj)}
                else:
                    hooks = {}
                psu = [psU.tile([DK + 1, BLK], f32, tag="psU",
                                name=f"ps_u{j}_{hp}_{p2}") for p2 in range(2)]

                def emit_u(iu, es_t):
                    for pi in range(2):
                        h = 2 * hp + pi
                        nc.tensor.matmul(
                            psu[pi][:],
                            vaug[:, iu, h, :],
                            es_t[:, pi * BLK : (pi + 1) * BLK],
                            start=(iu == 0),
                            stop=(iu == ST - 1),
                            skip_group_check=True,
                        )

                pending_tail = []
                pending_u = []
                for i in range(ST):
                    es = es_pool.tile([128, 2 * BLK], bf16, tag="es",
                                      name=f"es{j}_{hp}_{i}")
                    ps2 = psS.tile([128, 2 * BLK], f32, tag="psS",
                                   name=f"ps_s{j}_{hp}_{i}")
                    for pi in range(2):
                        bp = pi * 64
                        nc.tensor.matmul(
                            ps2[:, pi * BLK : (pi + 1) * BLK],
                            kt_[bp : bp + 64, mt, i * 128 : (i + 1) * 128],
                            qt[bp : bp + 64, mt, j * BLK : (j + 1) * BLK],
                            start=True,
                            stop=True,
                            skip_group_check=True,
                        )
                    if pattern[i] == "A":
                        nc.scalar.activation(es[:], ps2[:], Exp,
                                             bias=0.0, scale=0.125)
                        tail = None
                    else:
                        # op1 drains PSUM immediately (frees the psS slot for
                        # the PE); the SBUF-only chain tail is deferred until
                        # the lagged U matmul needs es.
                        scr = fe_pool.tile([128, 2 * BLK], f32, tag="scr",
                                           name=f"scr{j}_{hp}_{i}")
                        nc.vector.tensor_scalar(
                            scr[:], ps2[:], FE_SCALE, FE_MAGIC,
                            op0=Mult, op1=Add)

                        def tail(es=es, scr=scr, i=i):
                            phi = fe_pool.tile([128, 2 * BLK], i32, tag="phi",
                                               name=f"phi{j}_{hp}_{i}")
                            nc.vector.tensor_scalar(
                                phi[:], scr[:].bitcast(i32), 127, None,
                                op0=Band)
                            qsq = fe_pool.tile([128, 2 * BLK], f32, tag="qsq",
                                               name=f"qsq{j}_{hp}_{i}")
                            nc.vector.scalar_tensor_tensor(
                                qsq[:], phi[:], 128.0, phi[:],
                                op0=Sub, op1=Mult)
                            nc.vector.affine_then_add(
                                es[:].bitcast(u16), qsq[:], scr[:],
                                scale=FE_PA, bias=FE_K)
                    if i == 2 and deferred["norm"] is not None:
                        deferred["norm"]()
                        deferred["norm"] = None
                    if i == 5 and deferred["outproj"] is not None:
                        deferred["outproj"]()
                        deferred["outproj"] = None
                    if i in hooks:
                        hooks[i]()
                    pending_tail.append((i, tail))
                    pending_u.append((i, es))
                    if len(pending_tail) > TLAG:
                        _, tl = pending_tail.pop(0)
                        if tl is not None:
                            tl()
                    if len(pending_u) > ULAG:
                        emit_u(*pending_u.pop(0))
                for _, tl in pending_tail:
                    if tl is not None:
                        tl()
                for iu, es_t in pending_u:
                    emit_u(iu, es_t)
                def normalize(psu=psu, otj=otj, mt=mt, j=j, hp=hp):
                    # psu rows 0..63 = U; row 64 = denominators r. Copy r to
                    # partition 0 (plain DVE copy handles the shift), recip,
                    # broadcast on GpSimd, then normalize.
                    for pi in range(2):
                        bp = pi * 64
                        rrow = rpool.tile([1, BLK], f32, tag="r",
                                          name=f"rr{j}_{hp}_{pi}")
                        nc.vector.tensor_copy(rrow[:], psu[pi][DK : DK + 1, :])
                        rf = rpool.tile([1, BLK], f32, tag="rf",
                                        name=f"rf{j}_{hp}_{pi}")
                        nc.vector.reciprocal_approx_fast(rf[:], rrow[:])
                        rbc = upool.tile([DK, BLK], f32, tag="rb",
                                         name=f"rb{j}_{hp}_{pi}")
                        nc.gpsimd.partition_broadcast(rbc[:], rf[:])
                        nc.vector.tensor_mul(otj[bp : bp + 64, mt, :],
                                             psu[pi][0:DK, :], rbc[:])

                deferred["norm"] = normalize
                if j == 0 and hp == 0:
                    load_wo()
                if hp == 0 and ot_prev is not None:
                    deferred["outproj"] = (
                        lambda jp=j - 1, ot=ot_prev: emit_outproj(jp, ot, range(KT)))
            ot_prev = otj
        if deferred["norm"] is not None:
            deferred["norm"]()
        emit_outproj(NB - 1, ot_prev, range(KT))

    nc.compile()
    return nc


def get_program():
    if "nc" not in _CACHE:
        _CACHE["nc"] = _build_program()
    return _CACHE["nc"]


def make_core_inputs(query, key, value, Wq, bq, Wk, bk, Wv, bv, Wo, bo):
    """Build the 8 per-core input dicts (and the folded output bias)."""
    f = np.float32
    b16 = ml_dtypes.bfloat16
    in_maps = []
    for c in range(8):
        b, g = c // 2, c % 2
        hs = slice(g * LH, (g + 1) * LH)
        m = {
            "xq_t": np.ascontiguousarray(query[b].T).astype(b16),
            "xk_t": np.ascontiguousarray(key[b].T).astype(b16),
            "xv_t": np.ascontiguousarray(value[b].T).astype(b16),
            "wq": np.ascontiguousarray(
                Wq[hs].transpose(1, 0, 2).reshape(D, HK)).astype(b16),
            "wk": np.ascontiguousarray(
                Wk[hs].transpose(1, 0, 2).reshape(D, HK)).astype(b16),
            "wv": np.ascontiguousarray(
                Wv[hs].transpose(1, 0, 2).reshape(D, HK)).astype(b16),
            "wo": np.ascontiguousarray(Wo[g * HK : (g + 1) * HK, :]).astype(b16),
            # the score scale 1/sqrt(dk)=1/8 is applied at exp time
            "bq2": np.ascontiguousarray(
                bq[hs].reshape(HK).reshape(MT, 128).T, dtype=f
            ),
            "bk2": np.ascontiguousarray(
                bk[hs].reshape(HK).reshape(MT, 128).T, dtype=f
            ),
        }
        in_maps.append(m)
    bo_eff = (bv.reshape(H * DK).astype(np.float64) @ Wo.astype(np.float64)
              + bo.astype(np.float64)).astype(f)
    return in_maps, bo_eff


def combine_outputs(results, bo_eff):
    """results: list of 8 dicts with 'y_t' [D, S] bf16. Returns [B, S, D] f32."""
    out = np.empty((B, S, D), dtype=np.float32)
    for b in range(B):
        acc = (results[2 * b]["y_t"].astype(np.float32)
               + results[2 * b + 1]["y_t"].astype(np.float32))
        out[b] = acc.T + bo_eff[None, :]
    return out


def kernel(**inputs):
    from concourse.bass_utils import run_bass_kernel_spmd

    inputs = {k: np.asarray(v) for k, v in inputs.items()}
    nc = get_program()
    in_maps, bo_eff = make_core_inputs(
        inputs["query"], inputs["key"], inputs["value"],
        inputs["Wq"], inputs["bq"], inputs["Wk"], inputs["bk"],
        inputs["Wv"], inputs["bv"], inputs["Wo"], inputs["bo"],
    )
    res = run_bass_kernel_spmd(nc, in_maps, list(range(8)))
    return combine_outputs(res.results, bo_eff)


# revision 24
# speedup vs baseline: 1.1440x; 1.1440x over previous
"""Multi-head attention (B=4, S=2048, D=1024, H=16, dk=64) on 8 TRN2 NeuronCores.

Sharding: core c = (batch b = c//2, head-group g = c%2 of 8 heads).
Each core computes its head-group's attention output and the partial output
projection (Wo rows for its heads); the host sums the two partials per batch
and adds the (folded) output bias.

All-bf16 pipeline (fp8 was tried and abandoned: ~2.7% per-element
quantization noise does NOT average down through the diffuse softmax -- the
attention output's own magnitude shrinks identically -- so any fp8 stage
busts the 2e-2 budget; fp8 DoubleRow also measured no faster than bf16,
and fp8 matmuls pay a serialized ~88ns LDWEIGHTS that bf16's FWL hides).

Per-core stages:
  - Q/K/V projections: bf16 matmuls, PSUM drained with bias-add to bf16.
  - scores: bf16 64-row matmuls; head pairs on PE row groups 0/64 run
    concurrently. PSUM holds raw s = q.k (std ~8); exp applies the 1/8.
  - exp, split across engines (elementwise PSUM-drain rate is the second
    wall after the PE):
      'A' tiles: Scalar exact exp(s/8) -> bf16.
      'D' tiles: DVE 4-op accurate fast-exp (max err ~0.9%, median 0.2%):
        1. y = s*C0 + MAGIC  (magic-add rounds the bf16-bit index i into
           the f32 mantissa; MAGIC = 1.5*2^23 + 16256)
        2. phi = bits(y) & 127             (mantissa phase, int->float)
        3. q = (phi - 128) * phi           (scalar_tensor_tensor)
        4. es_bits = (q*PA + K) + y        (affine_then_add custom op;
           K = -1.5*2^23; PA*phi*(phi-128) = A*(phi-64)^2 - 10.98 is the
           2^f-vs-1+f mantissa correction; u16 out saturates underflow to 0)
  - attn @ V: bf16 [65,512] matmuls; V augmented with a trailing ones
    column so PSUM row 64 accumulates the softmax denominators.
  - normalize: r row (psum partition 64) copied to partition 0 on DVE,
    reciprocal_approx_fast, GpSimd broadcast, DVE multiply -> bf16 O^T.
  - output projection: bf16 matmuls; y drained to bf16 on Scalar, DMA out.
PSUM: psS pool 3x[128,1024] f32 (6 banks) backs the scores/exp pipeline,
the projection accumulators (two halves each) and the outproj psy; psU
2x[65,512] holds the U/denominator accumulators of the active unit.
V-bias and output bias are folded on the host: softmax rows sum to 1, so
bv contributes bv_cat @ Wo + bo to every row.
"""

import numpy as np
import ml_dtypes

B, S, D = 4, 2048, 1024
H, DK = 16, 64
LH = 8                 # heads per core
HK = LH * DK           # 512 (local concat dim)
BLK = 512              # Sq block size
NB = S // BLK          # 4
ST = S // 128          # 16 Skv tiles
KT = D // 128          # 8 contraction tiles over D
MT = HK // 128         # 4 m-tiles over local heads

# exp engine per kv tile i ('A' Scalar exact exp, 'D' DVE 4-op fast-exp),
# alternating by unit parity: A 216 / D 40 of 256 tiles total. D tiles lead
# each unit so their 4-op chains mature before the U matmuls (ULAG) need them.
EXP_PATTERNS = ["AAADDDAAAAAAAAAA",  # A=13, D=3
                "AAADDAAAAAAAAAAA"]  # A=14, D=2

# fast-exp consts: bf16 bits of exp(s/8): i = (s/8)*128*log2(e) + 16256
FE_SCALE = 184.6650053442343 / 8.0
FE_MAGIC = 12582912.0 + 16256.0
FE_PA = 10.9834 / 4096.0
FE_K = -12582912.0

_CACHE = {}


def _build_program():
    from contextlib import ExitStack
    import concourse.tile as tile
    from concourse import bacc, mybir

    f32 = mybir.dt.float32
    bf16 = mybir.dt.bfloat16
    i32 = mybir.dt.int32
    u16 = mybir.dt.uint16
    Exp = mybir.ActivationFunctionType.Exp
    Copy = mybir.ActivationFunctionType.Copy
    Mult = mybir.AluOpType.mult
    Add = mybir.AluOpType.add
    Band = mybir.AluOpType.bitwise_and
    Sub = mybir.AluOpType.subtract

    nc = bacc.Bacc("TRN2", target_bir_lowering=False, debug=False, num_devices=8)

    xq_d = nc.dram_tensor("xq_t", [D, S], bf16, kind="ExternalInput")
    xk_d = nc.dram_tensor("xk_t", [D, S], bf16, kind="ExternalInput")
    xv_d = nc.dram_tensor("xv_t", [D, S], bf16, kind="ExternalInput")
    wq_d = nc.dram_tensor("wq", [D, HK], bf16, kind="ExternalInput")
    wk_d = nc.dram_tensor("wk", [D, HK], bf16, kind="ExternalInput")
    wv_d = nc.dram_tensor("wv", [D, HK], bf16, kind="ExternalInput")
    wo_d = nc.dram_tensor("wo", [HK, D], bf16, kind="ExternalInput")
    bq_d = nc.dram_tensor("bq2", [128, MT], f32, kind="ExternalInput")
    bk_d = nc.dram_tensor("bk2", [128, MT], f32, kind="ExternalInput")
    y_d = nc.dram_tensor("y_t", [D, S], bf16, kind="ExternalOutput")

    with tile.TileContext(nc) as tc, ExitStack() as ctx:
        wpool = ctx.enter_context(tc.tile_pool(name="w", bufs=2))
        big = ctx.enter_context(tc.tile_pool(name="big", bufs=1))
        xs = ctx.enter_context(tc.tile_pool(name="xs", bufs=3))
        es_pool = ctx.enter_context(tc.tile_pool(name="es", bufs=6))
        fe_pool = ctx.enter_context(tc.tile_pool(name="fe", bufs=6))
        ot_pool = ctx.enter_context(tc.tile_pool(name="ot", bufs=2))
        rpool = ctx.enter_context(tc.tile_pool(name="r", bufs=2))
        upool = ctx.enter_context(tc.tile_pool(name="u", bufs=2))
        ypool = ctx.enter_context(tc.tile_pool(name="y", bufs=2))
        # PSUM: psS 3x[128,1024] f32 (6 banks) + psU 2x[65,512] (2) = 8
        psS = ctx.enter_context(tc.tile_pool(name="psS", bufs=3, space="PSUM"))
        psU = ctx.enter_context(tc.tile_pool(name="psU", bufs=2, space="PSUM"))

        bq_sb = big.tile([128, MT], f32)
        bk_sb = big.tile([128, MT], f32)
        nc.sync.dma_start(bq_sb[:], bq_d[:])
        nc.sync.dma_start(bk_sb[:], bk_d[:])
        qt = big.tile([128, MT, S], bf16)
        kt_ = big.tile([128, MT, S], bf16)
        # vaug[p, st, h, 0:DK] = V, [.., DK] = 1 (denominator column)
        vaug = big.tile([128, ST, LH, DK + 1], bf16)
        nc.vector.memset(vaug[:, :, :, :].bitcast(u16), 0x3F80)  # bf16 1.0

        def make_proj(x_dram, w_dram, bias_sb, dst, wname):
            """Returns (dma_block(j), mm_block(j)) emitters; loads weights now."""
            w_sb = wpool.tile([128, KT, HK], bf16, tag="w", name=f"w_{wname}")
            nc.sync.dma_start(w_sb[:], w_dram.ap().rearrange("(kt p) m -> p kt m", p=128))
            xts = {}

            def dma_block(j):
                xt = xs.tile([128, KT, BLK], bf16, tag="xs", name=f"xt_{wname}{j}")
                nc.sync.dma_start(
                    xt[:],
                    x_dram.ap()[:, j * BLK : (j + 1) * BLK]
                    .rearrange("(kt p) s -> p kt s", p=128),
                )
                xts[j] = xt

            def mm_block(j):
                xt = xts.pop(j)
                if wname == "v":
                    # V: x as stationary; drain [128, hk] kv-rows to vaug
                    for qp in range(2):
                        pt = psS.tile([128, 2 * BLK], f32, tag="psS",
                                      name=f"pp_v{j}_{qp}")
                        for kt in range(KT):
                            for t in range(2):
                                q = 2 * qp + t
                                nc.tensor.matmul(
                                    pt[:, t * BLK : (t + 1) * BLK],
                                    xt[:, kt, q * 128 : (q + 1) * 128],
                                    w_sb[:, kt, :],
                                    start=(kt == 0),
                                    stop=(kt == KT - 1),
                                    skip_group_check=True,
                                )
                        for t in range(2):
                            st = j * 4 + 2 * qp + t
                            nc.scalar.activation(
                                vaug[:, st, :, 0:DK],
                                pt[:, t * BLK : (t + 1) * BLK]
                                .rearrange("p (h k) -> p h k", h=LH),
                                Copy,
                            )
                else:
                    for mp in range(2):  # two mt-pairs share one psS slot
                        pt = psS.tile([128, 2 * BLK], f32, tag="psS",
                                      name=f"pp_{wname}{j}_{mp}")
                        for kt in range(KT):
                            for t in range(2):
                                mt = 2 * mp + t
                                nc.tensor.matmul(
                                    pt[:, t * BLK : (t + 1) * BLK],
                                    w_sb[:, kt, mt * 128 : (mt + 1) * 128],
                                    xt[:, kt, :],
                                    start=(kt == 0),
                                    stop=(kt == KT - 1),
                                    skip_group_check=True,
                                )
                        for t in range(2):
                            mt = 2 * mp + t
                            nc.vector.tensor_scalar_add(
                                dst[:, mt, j * BLK : (j + 1) * BLK],
                                pt[:, t * BLK : (t + 1) * BLK],
                                bias_sb[:, mt : mt + 1],
                            )

            return dma_block, mm_block

        # K fully first (scores of any unit span all kv blocks), then V/Q
        # block 0; remaining V/Q blocks are woven into the first attention
        # units so the exp engines start ~45us earlier.
        k_dma, k_mm = make_proj(xk_d, wk_d, bk_sb, kt_, "k")
        for j in range(NB):
            k_dma(j)
            k_mm(j)
        v_dma, v_mm = make_proj(xv_d, wv_d, None, None, "v")
        q_dma, q_mm = make_proj(xq_d, wq_d, bq_sb, qt, "q")
        v_dma(0)
        v_mm(0)
        q_dma(0)
        q_mm(0)

        state = {}

        def load_wo():
            wo_sb = wpool.tile([128, MT, D], bf16, tag="w")
            nc.sync.dma_start(
                wo_sb[:], wo_d.ap().rearrange("(kt p) m -> p kt m", p=128)
            )
            state["wo_sb"] = wo_sb

        def emit_outproj(jprev, ot_prev, mos):
            wo_sb = state["wo_sb"]
            for mo in mos:
                psy = psS.tile([128, 2 * BLK], f32, tag="psS",
                               name=f"psy{jprev}_{mo}")[:, 0:BLK]
                for kt in range(MT):
                    nc.tensor.matmul(
                        psy,
                        wo_sb[:, kt, mo * 128 : (mo + 1) * 128],
                        ot_prev[:, kt, :],
                        start=(kt == 0),
                        stop=(kt == MT - 1),
                        skip_group_check=True,
                    )
                ysb = ypool.tile([128, BLK], bf16, tag="y", name=f"ysb{jprev}_{mo}")
                nc.scalar.activation(ysb[:], psy, Copy)
                nc.sync.dma_start(
                    y_d[mo * 128 : (mo + 1) * 128,
                        jprev * BLK : (jprev + 1) * BLK], ysb[:]
                )

        ULAG = 4  # U matmuls trail the exp stream so D-tile chains mature
        TLAG = 2  # chain tails fire 2 tiles ahead of their U consumer
        ot_prev = None
        for j in range(NB):
            otj = ot_pool.tile([128, MT, BLK], bf16)
            for hp in range(LH // 2):
                mt = hp
                unit = j * 4 + hp
                pattern = EXP_PATTERNS[unit % 2]
                if j == 0 and hp == 0:
                    hooks = {1: lambda: v_dma(1), 3: lambda: v_mm(1),
                             5: lambda: v_dma(2), 7: lambda: v_mm(2),
                             9: lambda: v_dma(3), 11: lambda: v_mm(3)}
                elif j == 0:
                    hooks = {3: lambda jj=hp: q_dma(jj),
                             7: lambda jj=hp: q_mm(jj)}
                else:
                    hooks = {}
                psu = [psU.tile([DK + 1, BLK], f32, tag="psU",
                                name=f"ps_u{j}_{hp}_{p2}") for p2 in range(2)]

                def emit_u(iu, es_t):
                    for pi in range(2):
                        h = 2 * hp + pi
                        nc.tensor.matmul(
                            psu[pi][:],
                            vaug[:, iu, h, :],
                            es_t[:, pi * BLK : (pi + 1) * BLK],
                            start=(iu == 0),
                            stop=(iu == ST - 1),
                            skip_group_check=True,
                        )

                pending_tail = []
                pending_u = []
                for i in range(ST):
                    es = es_pool.tile([128, 2 * BLK], bf16, tag="es",
                                      name=f"es{j}_{hp}_{i}")
                    ps2 = psS.tile([128, 2 * BLK], f32, tag="psS",
                                   name=f"ps_s{j}_{hp}_{i}")
                    for pi in range(2):
                        bp = pi * 64
                        nc.tensor.matmul(
                            ps2[:, pi * BLK : (pi + 1) * BLK],
                            kt_[bp : bp + 64, mt, i * 128 : (i + 1) * 128],
                            qt[bp : bp + 64, mt, j * BLK : (j + 1) * BLK],
                            start=True,
                            stop=True,
                            skip_group_check=True,
                        )
                    if pattern[i] == "A":
                        nc.scalar.activation(es[:], ps2[:], Exp,
                                             bias=0.0, scale=0.125)
                        tail = None
                    else:
                        # op1 drains PSUM immediately (frees the psS slot for
                        # the PE); the SBUF-only chain tail is deferred until
                        # the lagged U matmul needs es.
                        scr = fe_pool.tile([128, 2 * BLK], f32, tag="scr",
                                           name=f"scr{j}_{hp}_{i}")
                        nc.vector.tensor_scalar(
                            scr[:], ps2[:], FE_SCALE, FE_MAGIC,
                            op0=Mult, op1=Add)

                        def tail(es=es, scr=scr, i=i):
                            phi = fe_pool.tile([128, 2 * BLK], i32, tag="phi",
                                               name=f"phi{j}_{hp}_{i}")
                            nc.vector.tensor_scalar(
                                phi[:], scr[:].bitcast(i32), 127, None,
                                op0=Band)
                            qsq = fe_pool.tile([128, 2 * BLK], f32, tag="qsq",
                                               name=f"qsq{j}_{hp}_{i}")
                            nc.vector.scalar_tensor_tensor(
                                qsq[:], phi[:], 128.0, phi[:],
                                op0=Sub, op1=Mult)
                            nc.vector.affine_then_add(
                                es[:].bitcast(u16), qsq[:], scr[:],
                                scale=FE_PA, bias=FE_K)
                    if i in hooks:
                        hooks[i]()
                    pending_tail.append((i, tail))
                    pending_u.append((i, es))
                    if len(pending_tail) > TLAG:
                        _, tl = pending_tail.pop(0)
                        if tl is not None:
                            tl()
                    if len(pending_u) > ULAG:
                        emit_u(*pending_u.pop(0))
                for _, tl in pending_tail:
                    if tl is not None:
                        tl()
                for iu, es_t in pending_u:
                    emit_u(iu, es_t)
                # psu rows 0..63 = U; row 64 = denominators r. Copy r to
                # partition 0 (plain DVE copy handles the shift), recip,
                # broadcast on GpSimd, then normalize.
                for pi in range(2):
                    bp = pi * 64
                    rrow = rpool.tile([1, BLK], f32, tag="r",
                                      name=f"rr{j}_{hp}_{pi}")
                    nc.vector.tensor_copy(rrow[:], psu[pi][DK : DK + 1, :])
                    rf = rpool.tile([1, BLK], f32, tag="rf",
                                    name=f"rf{j}_{hp}_{pi}")
                    nc.vector.reciprocal_approx_fast(rf[:], rrow[:])
                    rbc = upool.tile([DK, BLK], f32, tag="rb",
                                     name=f"rb{j}_{hp}_{pi}")
                    nc.gpsimd.partition_broadcast(rbc[:], rf[:])
                    nc.vector.tensor_mul(otj[bp : bp + 64, mt, :],
                                         psu[pi][0:DK, :], rbc[:])
                if j == 0 and hp == 0:
                    load_wo()
                if ot_prev is not None:
                    # spread the output projection of block j-1 across this
                    # block's units (2 columns each) to avoid psS ring bursts
                    emit_outproj(j - 1, ot_prev, range(2 * hp, 2 * hp + 2))
            ot_prev = otj
        emit_outproj(NB - 1, ot_prev, range(KT))

    nc.compile()
    return nc


def get_program():
    if "nc" not in _CACHE:
        _CACHE["nc"] = _build_program()
    return _CACHE["nc"]


def make_core_inputs(query, key, value, Wq, bq, Wk, bk, Wv, bv, Wo, bo):
    """Build the 8 per-core input dicts (and the folded output bias)."""
    f = np.float32
    b16 = ml_dtypes.bfloat16
    in_maps = []
    for c in range(8):
        b, g = c // 2, c % 2
        hs = slice(g * LH, (g + 1) * LH)
        m = {
            "xq_t": np.ascontiguousarray(query[b].T).astype(b16),
            "xk_t": np.ascontiguousarray(key[b].T).astype(b16),
            "xv_t": np.ascontiguousarray(value[b].T).astype(b16),
            "wq": np.ascontiguousarray(
                Wq[hs].transpose(1, 0, 2).reshape(D, HK)).astype(b16),
            "wk": np.ascontiguousarray(
                Wk[hs].transpose(1, 0, 2).reshape(D, HK)).astype(b16),
            "wv": np.ascontiguousarray(
                Wv[hs].transpose(1, 0, 2).reshape(D, HK)).astype(b16),
            "wo": np.ascontiguousarray(Wo[g * HK : (g + 1) * HK, :]).astype(b16),
            # the score scale 1/sqrt(dk)=1/8 is applied at exp time
            "bq2": np.ascontiguousarray(
                bq[hs].reshape(HK).reshape(MT, 128).T, dtype=f
            ),
            "bk2": np.ascontiguousarray(
                bk[hs].reshape(HK).reshape(MT, 128).T, dtype=f
            ),
        }
        in_maps.append(m)
    bo_eff = (bv.reshape(H * DK).astype(np.float64) @ Wo.astype(np.float64)
              + bo.astype(np.float64)).astype(f)
    return in_maps, bo_eff


def combine_outputs(results, bo_eff):
    """results: list of 8 dicts with 'y_t' [D, S] bf16. Returns [B, S, D] f32."""
    out = np.empty((B, S, D), dtype=np.float32)
    for b in range(B):
        acc = (results[2 * b]["y_t"].astype(np.float32)
               + results[2 * b + 1]["y_t"].astype(np.float32))
        out[b] = acc.T + bo_eff[None, :]
    return out


def kernel(**inputs):
    from concourse.bass_utils import run_bass_kernel_spmd

    inputs = {k: np.asarray(v) for k, v in inputs.items()}
    nc = get_program()
    in_maps, bo_eff = make_core_inputs(
        inputs["query"], inputs["key"], inputs["value"],
        inputs["Wq"], inputs["bq"], inputs["Wk"], inputs["bk"],
        inputs["Wv"], inputs["bv"], inputs["Wo"], inputs["bo"],
    )
    res = run_bass_kernel_spmd(nc, in_maps, list(range(8)))
    return combine_outputs(res.results, bo_eff)


# revision 25
# speedup vs baseline: 1.1647x; 1.0181x over previous
"""Multi-head attention (B=4, S=2048, D=1024, H=16, dk=64) on 8 TRN2 NeuronCores.

Sharding: core c = (batch b = c//2, head-group g = c%2 of 8 heads).
Each core computes its head-group's attention output and the partial output
projection (Wo rows for its heads); the host sums the two partials per batch
and adds the (folded) output bias.

All-bf16 pipeline (fp8 was tried and abandoned: ~2.7% per-element
quantization noise does NOT average down through the diffuse softmax -- the
attention output's own magnitude shrinks identically -- so any fp8 stage
busts the 2e-2 budget; fp8 DoubleRow also measured no faster than bf16,
and fp8 matmuls pay a serialized ~88ns LDWEIGHTS that bf16's FWL hides).

Per-core stages:
  - Q/K/V projections: bf16 matmuls, PSUM drained with bias-add to bf16.
  - scores: bf16 64-row matmuls; head pairs on PE row groups 0/64 run
    concurrently. PSUM holds raw s = q.k (std ~8); exp applies the 1/8.
  - exp, split across engines (elementwise PSUM-drain rate is the second
    wall after the PE):
      'A' tiles: Scalar exact exp(s/8) -> bf16.
      'D' tiles: DVE 4-op accurate fast-exp (max err ~0.9%, median 0.2%):
        1. y = s*C0 + MAGIC  (magic-add rounds the bf16-bit index i into
           the f32 mantissa; MAGIC = 1.5*2^23 + 16256)
        2. phi = bits(y) & 127             (mantissa phase, int->float)
        3. q = (phi - 128) * phi           (scalar_tensor_tensor)
        4. es_bits = (q*PA + K) + y        (affine_then_add custom op;
           K = -1.5*2^23; PA*phi*(phi-128) = A*(phi-64)^2 - 10.98 is the
           2^f-vs-1+f mantissa correction; u16 out saturates underflow to 0)
  - attn @ V: bf16 [65,512] matmuls; V augmented with a trailing ones
    column so PSUM row 64 accumulates the softmax denominators.
  - normalize: r row (psum partition 64) copied to partition 0 on DVE,
    reciprocal_approx_fast, GpSimd broadcast, DVE multiply -> bf16 O^T.
  - output projection: bf16 matmuls; y drained to bf16 on Scalar, DMA out.
PSUM: psS pool 3x[128,1024] f32 (6 banks) backs the scores/exp pipeline,
the projection accumulators (two halves each) and the outproj psy; psU
2x[65,512] holds the U/denominator accumulators of the active unit.
V-bias and output bias are folded on the host: softmax rows sum to 1, so
bv contributes bv_cat @ Wo + bo to every row.
"""

import numpy as np
import ml_dtypes

B, S, D = 4, 2048, 1024
H, DK = 16, 64
LH = 8                 # heads per core
HK = LH * DK           # 512 (local concat dim)
BLK = 512              # Sq block size
NB = S // BLK          # 4
ST = S // 128          # 16 Skv tiles
KT = D // 128          # 8 contraction tiles over D
MT = HK // 128         # 4 m-tiles over local heads

# exp engine per kv tile i ('A' Scalar exact exp, 'D' DVE 4-op fast-exp),
# alternating by unit parity: A 216 / D 40 of 256 tiles total. D tiles lead
# each unit so their 4-op chains mature before the U matmuls (ULAG) need them.
EXP_PATTERNS = ["AAADDDAAAAAAAAAA",  # A=13, D=3
                "AAADDAAAAAAAAAAA"]  # A=14, D=2

# fast-exp consts: bf16 bits of exp(s/8): i = (s/8)*128*log2(e) + 16256
FE_SCALE = 184.6650053442343 / 8.0
FE_MAGIC = 12582912.0 + 16256.0
FE_PA = 10.9834 / 4096.0
FE_K = -12582912.0

_CACHE = {}


def _build_program():
    from contextlib import ExitStack
    import concourse.tile as tile
    from concourse import bacc, mybir

    f32 = mybir.dt.float32
    bf16 = mybir.dt.bfloat16
    i32 = mybir.dt.int32
    u16 = mybir.dt.uint16
    Exp = mybir.ActivationFunctionType.Exp
    Copy = mybir.ActivationFunctionType.Copy
    Mult = mybir.AluOpType.mult
    Add = mybir.AluOpType.add
    Band = mybir.AluOpType.bitwise_and
    Sub = mybir.AluOpType.subtract

    nc = bacc.Bacc("TRN2", target_bir_lowering=False, debug=False, num_devices=8)

    xq_d = nc.dram_tensor("xq_t", [D, S], bf16, kind="ExternalInput")
    xk_d = nc.dram_tensor("xk_t", [D, S], bf16, kind="ExternalInput")
    xv_d = nc.dram_tensor("xv_t", [D, S], bf16, kind="ExternalInput")
    wq_d = nc.dram_tensor("wq", [D, HK], bf16, kind="ExternalInput")
    wk_d = nc.dram_tensor("wk", [D, HK], bf16, kind="ExternalInput")
    wv_d = nc.dram_tensor("wv", [D, HK], bf16, kind="ExternalInput")
    wo_d = nc.dram_tensor("wo", [HK, D], bf16, kind="ExternalInput")
    bq_d = nc.dram_tensor("bq2", [128, MT], f32, kind="ExternalInput")
    bk_d = nc.dram_tensor("bk2", [128, MT], f32, kind="ExternalInput")
    y_d = nc.dram_tensor("y_t", [D, S], bf16, kind="ExternalOutput")

    with tile.TileContext(nc) as tc, ExitStack() as ctx:
        wpool = ctx.enter_context(tc.tile_pool(name="w", bufs=3))
        big = ctx.enter_context(tc.tile_pool(name="big", bufs=1))
        xs = ctx.enter_context(tc.tile_pool(name="xs", bufs=3))
        es_pool = ctx.enter_context(tc.tile_pool(name="es", bufs=8))
        fe_pool = ctx.enter_context(tc.tile_pool(name="fe", bufs=6))
        ot_pool = ctx.enter_context(tc.tile_pool(name="ot", bufs=2))
        rpool = ctx.enter_context(tc.tile_pool(name="r", bufs=2))
        upool = ctx.enter_context(tc.tile_pool(name="u", bufs=2))
        ypool = ctx.enter_context(tc.tile_pool(name="y", bufs=2))
        # PSUM: psS 3x[128,1024] f32 (6 banks) + psU 2x[65,512] (2) = 8
        psS = ctx.enter_context(tc.tile_pool(name="psS", bufs=3, space="PSUM"))
        psU = ctx.enter_context(tc.tile_pool(name="psU", bufs=2, space="PSUM"))

        bq_sb = big.tile([128, MT], f32)
        bk_sb = big.tile([128, MT], f32)
        nc.sync.dma_start(bq_sb[:], bq_d[:])
        nc.sync.dma_start(bk_sb[:], bk_d[:])
        qt = big.tile([128, MT, S], bf16)
        kt_ = big.tile([128, MT, S], bf16)
        # vaug[p, st, h, 0:DK] = V, [.., DK] = 1 (denominator column)
        vaug = big.tile([128, ST, LH, DK + 1], bf16)
        nc.vector.memset(vaug[:, :, :, :].bitcast(u16), 0x3F80)  # bf16 1.0

        def make_proj(x_dram, w_dram, bias_sb, dst, wname):
            """Returns (dma_block(j), mm_block(j)) emitters; loads weights now."""
            w_sb = wpool.tile([128, KT, HK], bf16, tag="w", name=f"w_{wname}")
            nc.sync.dma_start(w_sb[:], w_dram.ap().rearrange("(kt p) m -> p kt m", p=128))
            xts = {}

            def dma_block(j):
                xt = xs.tile([128, KT, BLK], bf16, tag="xs", name=f"xt_{wname}{j}")
                nc.sync.dma_start(
                    xt[:],
                    x_dram.ap()[:, j * BLK : (j + 1) * BLK]
                    .rearrange("(kt p) s -> p kt s", p=128),
                )
                xts[j] = xt

            def mm_block(j):
                xt = xts.pop(j)
                if wname == "v":
                    # V: x as stationary; drain [128, hk] kv-rows to vaug
                    for qp in range(2):
                        pt = psS.tile([128, 2 * BLK], f32, tag="psS",
                                      name=f"pp_v{j}_{qp}")
                        for kt in range(KT):
                            for t in range(2):
                                q = 2 * qp + t
                                nc.tensor.matmul(
                                    pt[:, t * BLK : (t + 1) * BLK],
                                    xt[:, kt, q * 128 : (q + 1) * 128],
                                    w_sb[:, kt, :],
                                    start=(kt == 0),
                                    stop=(kt == KT - 1),
                                    skip_group_check=True,
                                )
                        for t in range(2):
                            st = j * 4 + 2 * qp + t
                            nc.scalar.activation(
                                vaug[:, st, :, 0:DK],
                                pt[:, t * BLK : (t + 1) * BLK]
                                .rearrange("p (h k) -> p h k", h=LH),
                                Copy,
                            )
                else:
                    for mp in range(2):  # two mt-pairs share one psS slot
                        pt = psS.tile([128, 2 * BLK], f32, tag="psS",
                                      name=f"pp_{wname}{j}_{mp}")
                        for kt in range(KT):
                            for t in range(2):
                                mt = 2 * mp + t
                                nc.tensor.matmul(
                                    pt[:, t * BLK : (t + 1) * BLK],
                                    w_sb[:, kt, mt * 128 : (mt + 1) * 128],
                                    xt[:, kt, :],
                                    start=(kt == 0),
                                    stop=(kt == KT - 1),
                                    skip_group_check=True,
                                )
                        for t in range(2):
                            mt = 2 * mp + t
                            nc.vector.tensor_scalar_add(
                                dst[:, mt, j * BLK : (j + 1) * BLK],
                                pt[:, t * BLK : (t + 1) * BLK],
                                bias_sb[:, mt : mt + 1],
                            )

            return dma_block, mm_block

        # K fully first (scores of any unit span all kv blocks), then V/Q
        # block 0; remaining V/Q blocks are woven into the first attention
        # units so the exp engines start ~45us earlier.
        k_dma, k_mm = make_proj(xk_d, wk_d, bk_sb, kt_, "k")
        v_dma, v_mm = make_proj(xv_d, wv_d, None, None, "v")
        q_dma, q_mm = make_proj(xq_d, wq_d, bq_sb, qt, "q")
        k_dma(0)
        k_mm(0)
        v_dma(0)
        v_mm(0)
        q_dma(0)
        q_mm(0)
        k_dma(1)

        state = {}

        def load_wo():
            wo_sb = wpool.tile([128, MT, D], bf16, tag="w")
            nc.sync.dma_start(
                wo_sb[:], wo_d.ap().rearrange("(kt p) m -> p kt m", p=128)
            )
            state["wo_sb"] = wo_sb

        def emit_outproj(jprev, ot_prev, mos):
            wo_sb = state["wo_sb"]
            for mo in mos:
                psy = psS.tile([128, 2 * BLK], f32, tag="psS",
                               name=f"psy{jprev}_{mo}")[:, 0:BLK]
                for kt in range(MT):
                    nc.tensor.matmul(
                        psy,
                        wo_sb[:, kt, mo * 128 : (mo + 1) * 128],
                        ot_prev[:, kt, :],
                        start=(kt == 0),
                        stop=(kt == MT - 1),
                        skip_group_check=True,
                    )
                ysb = ypool.tile([128, BLK], bf16, tag="y", name=f"ysb{jprev}_{mo}")
                nc.scalar.activation(ysb[:], psy, Copy)
                nc.sync.dma_start(
                    y_d[mo * 128 : (mo + 1) * 128,
                        jprev * BLK : (jprev + 1) * BLK], ysb[:]
                )

        ULAG = 6  # U matmuls trail the exp stream so D-tile chains mature
        TLAG = 3  # chain tails fire 3 tiles ahead of their U consumer
        ot_prev = None
        for j in range(NB):
            otj = ot_pool.tile([128, MT, BLK], bf16)
            for hp in range(LH // 2):
                mt = hp
                unit = j * 4 + hp
                pattern = EXP_PATTERNS[unit % 2]
                if j == 0 and hp == 0:
                    # scores(i) need K block i//4; U(i) (ULAG behind) needs
                    # V block i//4 -- weave the remaining projections in
                    hooks = {1: lambda: k_mm(1), 2: lambda: k_dma(2),
                             3: lambda: v_dma(1), 4: lambda: v_mm(1),
                             5: lambda: k_mm(2), 6: lambda: k_dma(3),
                             7: lambda: v_dma(2), 8: lambda: v_mm(2),
                             9: lambda: k_mm(3),
                             10: lambda: v_dma(3), 11: lambda: v_mm(3)}
                elif j == 0:
                    hooks = {3: lambda jj=hp: q_dma(jj),
                             7: lambda jj=hp: q_mm(jj)}
                else:
                    hooks = {}
                psu = [psU.tile([DK + 1, BLK], f32, tag="psU",
                                name=f"ps_u{j}_{hp}_{p2}") for p2 in range(2)]

                def emit_u(iu, es_t):
                    for pi in range(2):
                        h = 2 * hp + pi
                        nc.tensor.matmul(
                            psu[pi][:],
                            vaug[:, iu, h, :],
                            es_t[:, pi * BLK : (pi + 1) * BLK],
                            start=(iu == 0),
                            stop=(iu == ST - 1),
                            skip_group_check=True,
                        )

                pending_tail = []
                pending_u = []
                for i in range(ST):
                    es = es_pool.tile([128, 2 * BLK], bf16, tag="es",
                                      name=f"es{j}_{hp}_{i}")
                    ps2 = psS.tile([128, 2 * BLK], f32, tag="psS",
                                   name=f"ps_s{j}_{hp}_{i}")
                    for pi in range(2):
                        bp = pi * 64
                        nc.tensor.matmul(
                            ps2[:, pi * BLK : (pi + 1) * BLK],
                            kt_[bp : bp + 64, mt, i * 128 : (i + 1) * 128],
                            qt[bp : bp + 64, mt, j * BLK : (j + 1) * BLK],
                            start=True,
                            stop=True,
                            skip_group_check=True,
                        )
                    if pattern[i] == "A":
                        nc.scalar.activation(es[:], ps2[:], Exp,
                                             bias=0.0, scale=0.125)
                        tail = None
                    else:
                        # op1 drains PSUM immediately (frees the psS slot for
                        # the PE); the SBUF-only chain tail is deferred until
                        # the lagged U matmul needs es.
                        scr = fe_pool.tile([128, 2 * BLK], f32, tag="scr",
                                           name=f"scr{j}_{hp}_{i}")
                        nc.vector.tensor_scalar(
                            scr[:], ps2[:], FE_SCALE, FE_MAGIC,
                            op0=Mult, op1=Add)

                        def tail(es=es, scr=scr, i=i):
                            phi = fe_pool.tile([128, 2 * BLK], i32, tag="phi",
                                               name=f"phi{j}_{hp}_{i}")
                            nc.vector.tensor_scalar(
                                phi[:], scr[:].bitcast(i32), 127, None,
                                op0=Band)
                            qsq = fe_pool.tile([128, 2 * BLK], f32, tag="qsq",
                                               name=f"qsq{j}_{hp}_{i}")
                            nc.vector.scalar_tensor_tensor(
                                qsq[:], phi[:], 128.0, phi[:],
                                op0=Sub, op1=Mult)
                            nc.vector.affine_then_add(
                                es[:].bitcast(u16), qsq[:], scr[:],
                                scale=FE_PA, bias=FE_K)
                    if i in hooks:
                        hooks[i]()
                    pending_tail.append((i, tail))
                    pending_u.append((i, es))
                    if len(pending_tail) > TLAG:
                        _, tl = pending_tail.pop(0)
                        if tl is not None:
                            tl()
                    if len(pending_u) > ULAG:
                        emit_u(*pending_u.pop(0))
                for _, tl in pending_tail:
                    if tl is not None:
                        tl()
                for iu, es_t in pending_u:
                    emit_u(iu, es_t)
                # psu rows 0..63 = U; row 64 = denominators r. Copy r to
                # partition 0 (plain DVE copy handles the shift), recip,
                # broadcast on GpSimd, then normalize.
                for pi in range(2):
                    bp = pi * 64
                    rrow = rpool.tile([1, BLK], f32, tag="r",
                                      name=f"rr{j}_{hp}_{pi}")
                    nc.vector.tensor_copy(rrow[:], psu[pi][DK : DK + 1, :])
                    rf = rpool.tile([1, BLK], f32, tag="rf",
                                    name=f"rf{j}_{hp}_{pi}")
                    nc.vector.reciprocal_approx_fast(rf[:], rrow[:])
                    rbc = upool.tile([DK, BLK], f32, tag="rb",
                                     name=f"rb{j}_{hp}_{pi}")
                    nc.gpsimd.partition_broadcast(rbc[:], rf[:])
                    nc.vector.tensor_mul(otj[bp : bp + 64, mt, :],
                                         psu[pi][0:DK, :], rbc[:])
                if j == 0 and hp == 0:
                    load_wo()
                if ot_prev is not None:
                    # spread the output projection of block j-1 across this
                    # block's units (2 columns each) to avoid psS ring bursts
                    emit_outproj(j - 1, ot_prev, range(2 * hp, 2 * hp + 2))
            ot_prev = otj
        emit_outproj(NB - 1, ot_prev, range(KT))

    nc.compile()
    return nc


def get_program():
    if "nc" not in _CACHE:
        _CACHE["nc"] = _build_program()
    return _CACHE["nc"]


def make_core_inputs(query, key, value, Wq, bq, Wk, bk, Wv, bv, Wo, bo):
    """Build the 8 per-core input dicts (and the folded output bias)."""
    f = np.float32
    b16 = ml_dtypes.bfloat16
    in_maps = []
    for c in range(8):
        b, g = c // 2, c % 2
        hs = slice(g * LH, (g + 1) * LH)
        m = {
            "xq_t": np.ascontiguousarray(query[b].T).astype(b16),
            "xk_t": np.ascontiguousarray(key[b].T).astype(b16),
            "xv_t": np.ascontiguousarray(value[b].T).astype(b16),
            "wq": np.ascontiguousarray(
                Wq[hs].transpose(1, 0, 2).reshape(D, HK)).astype(b16),
            "wk": np.ascontiguousarray(
                Wk[hs].transpose(1, 0, 2).reshape(D, HK)).astype(b16),
            "wv": np.ascontiguousarray(
                Wv[hs].transpose(1, 0, 2).reshape(D, HK)).astype(b16),
            "wo": np.ascontiguousarray(Wo[g * HK : (g + 1) * HK, :]).astype(b16),
            # the score scale 1/sqrt(dk)=1/8 is applied at exp time
            "bq2": np.ascontiguousarray(
                bq[hs].reshape(HK).reshape(MT, 128).T, dtype=f
            ),
            "bk2": np.ascontiguousarray(
                bk[hs].reshape(HK).reshape(MT, 128).T, dtype=f
            ),
        }
        in_maps.append(m)
    bo_eff = (bv.reshape(H * DK).astype(np.float64) @ Wo.astype(np.float64)
              + bo.astype(np.float64)).astype(f)
    return in_maps, bo_eff


def combine_outputs(results, bo_eff):
    """results: list of 8 dicts with 'y_t' [D, S] bf16. Returns [B, S, D] f32."""
    out = np.empty((B, S, D), dtype=np.float32)
    for b in range(B):
        acc = (results[2 * b]["y_t"].astype(np.float32)
               + results[2 * b + 1]["y_t"].astype(np.float32))
        out[b] = acc.T + bo_eff[None, :]
    return out


def kernel(**inputs):
    from concourse.bass_utils import run_bass_kernel_spmd

    inputs = {k: np.asarray(v) for k, v in inputs.items()}
    nc = get_program()
    in_maps, bo_eff = make_core_inputs(
        inputs["query"], inputs["key"], inputs["value"],
        inputs["Wq"], inputs["bq"], inputs["Wk"], inputs["bk"],
        inputs["Wv"], inputs["bv"], inputs["Wo"], inputs["bo"],
    )
    res = run_bass_kernel_spmd(nc, in_maps, list(range(8)))
    return combine_outputs(res.results, bo_eff)
